# revision 24
# baseline (speedup 1.0000x reference)
"""Trainium2 Bass kernel for DescriptorMatcher (mutual nearest neighbor).

Problem: desc0 [B,N,D], desc1 [B,M,D] (B=4, N=M=8192, D=128, fp32):
    sim     = desc0 @ desc1^T                      [B,N,M]
    score0  = max_m sim;  match01 = argmax_m sim   [B,N]
    match10 = argmax_n sim                         [B,M]
    valid   = (match10[match01[n]] == n) & (score0 > 0.1)
returns (match01, score0, valid).

Sharding: 8 cores = 4 batches x 2 row-halves (4096 rows each).

Phase 1 (per core): the bulk O(N*M) work in a fast approximate domain.
  Inputs are pre-rounded on host to 11 mantissa bits, which makes the PE's
  float32r mode (1 cycle/row, 4x faster than float32) bit-exactly
  reproducible on host. Per 128-row tile:
    PE:   16 f32r matmuls -> PSUM [128,2048] x4           (sim tile, fp32)
    ACT:  copy PSUM -> SBUF row buffer as fp16            (scan domain)
    DVE:  colacc = max(colacc, row)             (fp16, 2x mode)
    DVE:  binary-tree max over each 256-wide chunk -> CM [128,32] fp16
  (GPSIMD/Pool cannot run TensorTensor on real HW — codegen rejects the
  opcode — so DVE carries both scans and is the pacing engine.)
  The per-column max stays partition-partial: colacc [128, M] is DMA'd out
  whole and the host folds the 128 partitions (and the core pair) in fp16.

Phase 2 (per core): exact rescue. Host groups rows by candidate chunks
  (all chunks whose fp16 CM is within GUARD=0.3 of the row's best — covers
  the f32r+fp16 noise, <=0.03, with 10x margin). For each 128-slot subtile,
  recompute sim[rows, chunk] in full fp32 (dt.float32 matmul, unrounded
  inputs) and take Max + MaxIndex directly on PSUM. Host picks the best
  candidate chunk per row by exact fp32 value (ties -> lowest chunk, matching
  argmax first-occurrence semantics).

Outputs: score0/match01 are exact fp32 (phase 2). valid uses the fp16 scan
domain consistently on both sides (score_scan == colmax_scan[match01]); the
only mismatches vs the fp32 reference are near-exact-tie columns (~1e-3 of
rows on gaussian data, far under the 2e-2 gate).

Rows overflowing a chunk group's padded capacity (needs >256 of ~140
expected; +9 sigma) fall back to an exact host recompute of that row.
"""

import numpy as np

import concourse.bass as bass  # noqa: F401  (bass must import before tile)
import concourse.mybir as mybir
import concourse.tile as tile
from concourse import bacc, bass_isa

B, N, M, D = 4, 8192, 8192, 128
NCORES = 8
HALF = N // 2          # rows per core
NT = HALF // 128       # 32 n-tiles per core
CW = 1024              # input-DMA chunk width
CHW = 256              # phase-2 chunk width
NCH = M // CHW         # 32 chunks per row
GUARD = 0.3            # chunk-candidate guard band (noise <= ~0.03)
PAD2 = 256             # phase-2 slots per chunk group (expected ~140 + 9sd)
NST2 = NCH * PAD2 // 128   # 64 phase-2 subtiles
F16_MIN = -65504.0


def _round11(x):
    """Round fp32 to 11 mantissa bits (lossless through the f32r PE path)."""
    xi = np.ascontiguousarray(x).view(np.uint32)
    return ((xi.astype(np.uint64) + np.uint32(1 << 11)).astype(np.uint32)
            & np.uint32(0xFFFFF000)).view(np.float32)


def _build1():
    f32, f32r, f16 = mybir.dt.float32, mybir.dt.float32r, mybir.dt.float16
    mx = mybir.AluOpType.max
    nc = bacc.Bacc("TRN2", target_bir_lowering=False, debug=False,
                   num_devices=NCORES)
    at = nc.dram_tensor("at", [D, HALF], f32r, kind="ExternalInput").ap()
    bt = nc.dram_tensor("bt", [D, M], f32r, kind="ExternalInput").ap()
    cm_o = nc.dram_tensor("cm", [128, NT * NCH], f16, kind="ExternalOutput").ap()
    colp_o = nc.dram_tensor("colp", [128, M], f16, kind="ExternalOutput").ap()

    with tile.TileContext(nc) as tc:
        with tc.tile_pool(name="big", bufs=1) as big, \
             tc.tile_pool(name="rows", bufs=3) as rows, \
             tc.tile_pool(name="scr", bufs=2) as scr, \
             tc.tile_pool(name="ps", bufs=2, space="PSUM") as ps:
            atb = big.tile([128, HALF], f32r, name="atb")
            btb = big.tile([128, M], f32r, name="btb")
            # btb is tile 0's critical path: a thin slice of atb first, then
            # all of btb, then the rest of atb
            nc.sync.dma_start(atb[:, 0:CW], at[:, 0:CW])
            for c in range(0, M, CW):
                nc.sync.dma_start(btb[:, c:c + CW], bt[:, c:c + CW])
            for c in range(CW, HALF, CW):
                nc.sync.dma_start(atb[:, c:c + CW], at[:, c:c + CW])
            colacc = big.tile([128, M], f16, name="colacc")
            cm_all = big.tile([128, NT * NCH], f16, name="cm_all")
            for t in range(NT):
                row = rows.tile([128, M], f16, tag="row", name="row")
                for c in range(4):
                    pt = ps.tile([128, 2048], f32, tag="pt", name="pt")
                    for j in range(4):
                        mlo = c * 2048 + j * 512
                        nc.tensor.matmul(pt[:, j * 512:(j + 1) * 512],
                                         atb[:, t * 128:(t + 1) * 128],
                                         btb[:, mlo:mlo + 512],
                                         start=True, stop=True)
                    nc.scalar.copy(row[:, c * 2048:(c + 1) * 2048], pt[:])
                # column max accumulate (fp16, DVE 2x mode); first tile seeds
                # via copy (TensorCopy runs 4x). Tile 0 is the pipeline head:
                # split its DVE work per 2048-quarter so it starts as soon as
                # the first ACT quarter lands instead of after all four.
                sA = scr.tile([128, 4096], f16, tag="sA", name="sA")
                sB = scr.tile([128, 2048], f16, tag="sB", name="sB")
                src = row[:].rearrange("p (c w) -> p c w", w=CHW)
                w = CHW // 2
                dst = sA[:, 0:NCH * w].rearrange("p (c w) -> p c w", w=w)
                if t == 0:
                    cpq = NCH // 4           # chunks per 2048-quarter
                    for q in range(4):
                        nc.vector.tensor_copy(colacc[:, q * 2048:(q + 1) * 2048],
                                              row[:, q * 2048:(q + 1) * 2048])
                        nc.vector.tensor_tensor(
                            dst[:, q * cpq:(q + 1) * cpq],
                            src[:, q * cpq:(q + 1) * cpq, 0:w],
                            src[:, q * cpq:(q + 1) * cpq, w:2 * w], op=mx)
                else:
                    nc.vector.tensor_tensor(colacc[:], colacc[:], row[:], op=mx)
                    nc.vector.tensor_tensor(dst, src[:, :, 0:w],
                                            src[:, :, w:2 * w], op=mx)
                cur, bufs = dst, (sB, sA)
                i = 0
                while w > 2:
                    w //= 2
                    dst = bufs[i % 2][:, 0:NCH * w].rearrange(
                        "p (c w) -> p c w", w=w)
                    nc.vector.tensor_tensor(dst, cur[:, :, 0:w],
                                            cur[:, :, w:2 * w], op=mx)
                    cur, i = dst, i + 1
                dst = cm_all[:, t * NCH:(t + 1) * NCH].rearrange(
                    "p (c w) -> p c w", w=1)
                nc.vector.tensor_tensor(dst, cur[:, :, 0:1], cur[:, :, 1:2],
                                        op=mx)
            nc.sync.dma_start(cm_o[:], cm_all[:])
            nc.sync.dma_start(colp_o[:], colacc[:])
    nc.compile()
    return nc


def _build2():
    f32, u32 = mybir.dt.float32, mybir.dt.uint32
    nc = bacc.Bacc("TRN2", target_bir_lowering=False, debug=False,
                   num_devices=NCORES)
    at2 = nc.dram_tensor("at2", [D, NCH * PAD2], f32, kind="ExternalInput").ap()
    bt2 = nc.dram_tensor("bt2", [D, M], f32, kind="ExternalInput").ap()
    mx_o = nc.dram_tensor("mx8", [128, NST2 * 8], f32, kind="ExternalOutput").ap()
    ix_o = nc.dram_tensor("ix8", [128, NST2 * 8], u32, kind="ExternalOutput").ap()
    with tile.TileContext(nc) as tc:
        with tc.tile_pool(name="big", bufs=1) as big, \
             tc.tile_pool(name="ps", bufs=8, space="PSUM") as ps:
            a2b = big.tile([128, NCH * PAD2], f32, name="a2b")
            btb = big.tile([128, M], f32, name="btb")
            # the first subtiles' inputs first, then the rest
            nc.sync.dma_start(a2b[:, 0:CW], at2[:, 0:CW])
            for c in range(0, M, CW):
                nc.sync.dma_start(btb[:, c:c + CW], bt2[:, c:c + CW])
            for c in range(CW, NCH * PAD2, CW):
                nc.sync.dma_start(a2b[:, c:c + CW], at2[:, c:c + CW])
            mx8 = big.tile([128, NST2 * 8], f32, name="mx8")
            ix8 = big.tile([128, NST2 * 8], u32, name="ix8")
            KP = PAD2 // 128
            for st in range(NST2):
                pt = ps.tile([128, CHW], f32, tag="pt", name="pt")
                nc.tensor.matmul(pt[:],
                                 a2b[:, st * 128:(st + 1) * 128],
                                 btb[:, (st // KP) * CHW:(st // KP + 1) * CHW],
                                 start=True, stop=True)
                nc.vector.max(mx8[:, st * 8:(st + 1) * 8], pt[:])
                nc.vector.max_index(ix8[:, st * 8:(st + 1) * 8],
                                    mx8[:, st * 8:(st + 1) * 8], pt[:])
            nc.sync.dma_start(mx_o[:], mx8[:])
            nc.sync.dma_start(ix_o[:], ix8[:])
    nc.compile()
    return nc


_cached = None


def _make_exec(nc):
    import jax
    from jax.sharding import Mesh, PartitionSpec
    from jax.experimental.shard_map import shard_map
    from concourse import bass2jax
    from concourse.bass2jax import _bass_exec_p

    partition_name = nc.partition_id_tensor.name if nc.partition_id_tensor else None
    in_names, out_names, out_avals, out_shapes = [], [], [], []
    for alloc in nc.m.functions[0].allocations:
        if not isinstance(alloc, mybir.MemoryLocationSet):
            continue
        name = alloc.memorylocations[0].name
        if alloc.kind == "ExternalInput":
            if name != partition_name:
                in_names.append(name)
        elif alloc.kind == "ExternalOutput":
            shape = tuple(alloc.tensor_shape)
            dtype = mybir.dt.np(alloc.dtype)
            out_names.append(name)
            out_shapes.append((shape, dtype))
            out_avals.append(jax.core.ShapedArray(shape, dtype))
    n_params = len(in_names)
    n_outs = len(out_names)
    all_in_names = in_names + out_names
    if partition_name is not None:
        all_in_names = all_in_names + [partition_name]

    def _body(*args):
        operands = list(args)
        if partition_name is not None:
            operands.append(bass2jax.partition_id_tensor())
        outs = _bass_exec_p.bind(
            *operands, out_avals=tuple(out_avals), in_names=tuple(all_in_names),
            out_names=tuple(out_names), lowering_input_output_aliases=(),
            sim_require_finite=True, sim_require_nnan=True, nc=nc)
        return tuple(outs)

    devices = jax.devices()[:NCORES]
    mesh = Mesh(np.asarray(devices), ("core",))
    in_specs = (PartitionSpec("core"),) * (n_params + n_outs)
    out_specs = (PartitionSpec("core"),) * n_outs
    fn = jax.jit(shard_map(_body, mesh=mesh, in_specs=in_specs,
                           out_specs=out_specs, check_rep=False),
                 keep_unused=True)
    return {"fn": fn, "in_names": in_names, "out_names": out_names,
            "out_shapes": out_shapes, "nc": nc}


def _run(ex, ins):
    """ins: dict name -> [NCORES, *shape]; returns dict name -> [NCORES, *shape]."""
    concat_in = [np.ascontiguousarray(ins[n].reshape(-1, *ins[n].shape[2:]))
                 for n in ex["in_names"]]
    concat_zeros = [np.zeros((NCORES * s[0], *s[1:]), dt)
                    for (s, dt) in ex["out_shapes"]]
    out_arrs = ex["fn"](*concat_in, *concat_zeros)
    return {name: np.asarray(out_arrs[i]).reshape(NCORES, *ex["out_shapes"][i][0])
            for i, name in enumerate(ex["out_names"])}


def kernel(desc0, desc1):
    global _cached
    desc0 = np.asarray(desc0, dtype=np.float32)
    desc1 = np.asarray(desc1, dtype=np.float32)
    assert desc0.shape == (B, N, D) and desc1.shape == (B, M, D)

    if _cached is None:
        _cached = (_make_exec(_build1()), _make_exec(_build2()))
    ex1, ex2 = _cached

    a_slab = np.stack([desc0[b, h * HALF:(h + 1) * HALF]
                       for b in range(B) for h in range(2)])      # [8,4096,128]
    bt_all = np.stack([desc1[b].transpose(1, 0)
                       for b in range(B) for h in range(2)])      # [8,128,8192]
    at_all = a_slab.transpose(0, 2, 1)                            # [8,128,4096]
    bt_hi = _round11(bt_all)

    r1 = _run(ex1, {"at": _round11(at_all), "bt": bt_hi})

    # host glue: scan-domain score + guard-banded chunk candidates
    cm = r1["cm"].reshape(NCORES, 128, NT, NCH).transpose(0, 2, 1, 3) \
                 .reshape(NCORES, HALF, NCH)                      # fp16
    score_s = cm.max(axis=2)                                      # fp16 [8,4096]
    cand = cm.astype(np.float32) >= \
        (score_s.astype(np.float32) - GUARD)[:, :, None]          # [8,4096,16]

    at2 = np.zeros((NCORES, D, NCH * PAD2), np.float32)
    cand_rows = []                                                # per core
    overflow = []                                                 # (core, row)
    for core in range(NCORES):
        rows_l, chunks_l, slots_l = [], [], []
        for g in range(NCH):
            rows_g = np.nonzero(cand[core, :, g])[0]
            if len(rows_g) > PAD2:
                overflow.extend((core, r) for r in rows_g[PAD2:])
                rows_g = rows_g[:PAD2]
            slots = g * PAD2 + np.arange(len(rows_g))
            at2[core][:, slots] = a_slab[core][rows_g].T
            rows_l.append(rows_g)
            chunks_l.append(np.full(len(rows_g), g))
            slots_l.append(slots)
        cand_rows.append((np.concatenate(rows_l), np.concatenate(chunks_l),
                          np.concatenate(slots_l)))

    r2 = _run(ex2, {"at2": at2, "bt2": bt_all})
    mxs = r2["mx8"].reshape(NCORES, 128, NST2, 8)[:, :, :, 0]     # [8,128,48]
    ixs = r2["ix8"].reshape(NCORES, 128, NST2, 8)[:, :, :, 0]

    match01 = np.empty((B, N), dtype=np.int32)
    score0 = np.empty((B, N), dtype=np.float32)
    valid = np.empty((B, N), dtype=bool)
    # fold 128 partitions and the core pair on host (fp16 max chain is exact)
    colmax = r1["colp"].reshape(B, 2 * 128, M).max(axis=1)        # fp16 [B,M]

    for core in range(NCORES):
        b, h = divmod(core, 2)
        rows_c, chunks_c, slots_c = cand_rows[core]
        vals = mxs[core][slots_c % 128, slots_c // 128]
        within = ixs[core][slots_c % 128, slots_c // 128].astype(np.int64)
        vmat = np.full((HALF, NCH), -np.inf, np.float32)
        wmat = np.zeros((HALF, NCH), np.int64)
        vmat[rows_c, chunks_c] = vals
        wmat[rows_c, chunks_c] = within
        gbest = vmat.argmax(axis=1)                  # first max -> lowest chunk
        ar = np.arange(HALF)
        m = gbest * CHW + wmat[ar, gbest]
        s = vmat[ar, gbest]
        sel = slice(h * HALF, (h + 1) * HALF)
        match01[b, sel] = m.astype(np.int32)
        score0[b, sel] = s
        valid[b, sel] = (s > 0.1) & (score_s[core] == colmax[b][m])

    for core, row in overflow:                                    # ~never taken
        b, h = divmod(core, 2)
        simrow = a_slab[core][row] @ desc1[b].T
        n = h * HALF + row
        match01[b, n] = int(simrow.argmax())
        score0[b, n] = simrow.max()
        valid[b, n] = (score0[b, n] > 0.1) & \
                      (score_s[core][row] == colmax[b][match01[b, n]])

    return match01, score0, valid
